# revision 18
# baseline (speedup 1.0000x reference)
"""Multi-head causal self-attention block for Trainium2, data-parallel over 8 cores.

Reference computation (per batch b of x [B=32, T=1024, C=384]):
    qkv = x @ W_attn;  q,k,v heads (H=6, D=64)
    y   = softmax(causal(q k^T / sqrt(D))) @ v
    out = y @ W_proj + b_proj
Sharding: batch dim 32 -> 4 per core, weights replicated, no collectives.

v6 design (evolves v3..v5; bf16 matmul operands, f32 psum accumulation):
  - Every dma_start costs ~600ns of ISSUING-sequencer occupancy, so DMAs are
    batched hard: x loads are 2 DMAs/batch, x^T and y^T crossbar transposes
    are 2 calls/batch each (covering 4 token tiles at a time), and output
    stores are 2 gpsimd SWDGE DMAs/batch (keeping the sync queue short).
  - x^T / y^T live in T-MAJOR chunk layout (chunk index = t*3 + c) so a
    4-tile crossbar call writes one contiguous [128, 12, 128] region; the
    PE does no transposes at all and the DVE does no transpose evictions.
  - x is cast f32->bf16 on the DVE in 768-col quarters (gpsimd tensor ops
    are ~4x slower and its in-order queue must stay clear for the causal
    affine_selects).
  - q^T,k^T per head-pair [128, T] (W_attn slices stationary); v natural
    [tok, 384] per k-tile with a bf16 ones column per head (softmax
    denominator rides the AV matmul for free).
  - scores transposed s^T[k,q] per 512-wide q chunk, exp on ACT (scale=1/8)
    into bf16 pT; causal diagonal fixed by gpsimd affine_select.
  - AV in NATURAL layout: out[q, d+1] accumulated per 128-q chunk; softmax
    normalization = strided DVE reciprocal + broadcast-AP multiplies.
  - proj runs as deferred filler work (with a readiness delay) so the y^T
    crossbar latency hides behind the next q-chunk's score phase; bias is
    added on the DVE eviction into a 4-tile output buffer.
  - VIRTUAL-CLOCK INTERLEAVE: emission tracks estimated cumulative PE and
    ACT busy-ns; filler chunks (next batch's cast/xbar/v/qk, deferred
    proj) are inserted whenever the PE stream would fall behind the ACT
    stream, keeping the in-order PE queue dense so the tensor engine
    stays ramped at its top p-state. Fillers carry deadline tags (global
    qc index; force-emitted when their consumer phase begins) and a
    readiness gate (min ACT-clock before emission). Per-batch prep is
    split into an "early" part (needed before the batch's qc0 attention)
    and a "late" part (only needed by qc1), widening the filler window.
"""

import sys

if "/opt/trn_rl_repo" not in sys.path:
    sys.path.insert(0, "/opt/trn_rl_repo")

import numpy as np

B, T, C = 32, 1024, 384
H, D = 6, 64
NCORES = 8
BPC = B // NCORES          # batches per core
NPAIR = H // 2             # head pairs
TT = T // 128              # token tiles per batch (8)
QC = T // 512              # q chunks per batch (2)
VSTRIDE = H * (D + 1)      # 390: per-token-tile v_aug row width

VM_MARGIN = 500.0          # ns of PE lead to maintain over ACT
VM_CAP = 1500.0            # max bankable PE lead (queue depth model)
PROJ_DELAY = 2500.0        # ns of ACT-clock before a deferred proj is ready

_nc_cache = {}


def _build_nc():
    import concourse.mybir as mybir
    from concourse import bacc
    from concourse.tile import TileContext

    f32 = mybir.dt.float32
    bf16 = mybir.dt.bfloat16
    Exp = mybir.ActivationFunctionType.Exp
    GE = mybir.AluOpType.is_ge

    nc = bacc.Bacc("TRN2", target_bir_lowering=False, debug=False, num_devices=NCORES)

    x_d = nc.declare_dram_parameter("x", [BPC, T, C], f32, isOutput=False)
    wa_d = nc.declare_dram_parameter("W_attn", [C, 3 * C], f32, isOutput=False)
    wp_d = nc.declare_dram_parameter("W_proj", [C, C], f32, isOutput=False)
    bp_d = nc.declare_dram_parameter("b_proj", [C], f32, isOutput=False)
    out_d = nc.declare_dram_parameter("out", [BPC, T, C], f32, isOutput=True)

    # virtual clocks (ns) for PE / ACT emission balancing
    est = {"pe": 0.0, "act": 0.0}

    def MM(n):                      # matmul cost, cols n
        return n / 2.4 + 10.0

    def EXPC(n):                    # ACT exp cost, free-elems n
        return n * 1.0 + 250.0

    def bump_pe(cost):
        est["pe"] = min(est["pe"] + cost, est["act"] + VM_CAP)

    def tc_off(t, c):               # t-major chunk offset into xT / yT
        return (t * 3 + c) * 128

    with TileContext(nc) as tc:
        with (
            tc.tile_pool(name="const", bufs=1) as const,
            tc.tile_pool(name="xf", bufs=2) as xfp,
            tc.tile_pool(name="xb", bufs=2) as xbp,
            tc.tile_pool(name="xT", bufs=2) as xTp,
            tc.tile_pool(name="qk", bufs=2) as qkp,
            tc.tile_pool(name="vb", bufs=2) as vbp,
            tc.tile_pool(name="pT", bufs=2) as pTp,
            tc.tile_pool(name="yn", bufs=2) as ynp,
            tc.tile_pool(name="yT", bufs=2) as yTp,
            tc.tile_pool(name="rc", bufs=4) as rcp,
            tc.tile_pool(name="osb", bufs=2) as osbp,
            tc.tile_pool(name="psA", bufs=2, space="PSUM") as psA,   # 1 bank each
            tc.tile_pool(name="psS", bufs=2, space="PSUM") as psS,   # 2 banks each
            tc.tile_pool(name="psY", bufs=1, space="PSUM") as psY,   # 2 banks
        ):
            def load(b):
                """x[b] f32 DRAM -> SBUF staging, two 4-tile DMAs."""
                xf = xfp.tile([128, TT * C], f32, tag="xf", name=f"xf{b}")
                for hf in range(2):
                    nc.sync.dma_start(
                        out=xf[:, hf * 4 * C:(hf + 1) * 4 * C]
                        .rearrange("p (t c) -> p t c", t=4),
                        in_=x_d[b, hf * 512:(hf + 1) * 512, :]
                        .rearrange("(t p) c -> p t c", p=128),
                    )
                return xf

            # ---- prologue: x0 first (longest chain), then weight DMAs;
            # weight CASTS are emitted inside early0 after the first x
            # casts so the DVE starts the x chain as soon as data lands ----
            xf0 = load(0)

            wa_sb = []
            wp_sb = []
            wstg = []
            for c in range(3):
                wf = const.tile([128, 4 * C], f32, tag=f"wf{c}")
                nc.scalar.dma_start(out=wf[:, 0: 3 * C],
                                    in_=wa_d[c * 128:(c + 1) * 128, :])
                nc.scalar.dma_start(out=wf[:, 3 * C: 4 * C],
                                    in_=wp_d[c * 128:(c + 1) * 128, :])
                wstg.append(wf)
            for c in range(3):
                w = const.tile([128, 3 * C], bf16, tag=f"wa{c}")
                wa_sb.append(w)
                p = const.tile([128, C], bf16, tag=f"wp{c}")
                wp_sb.append(p)
            b_bc = const.tile([128, C], f32, tag="bbc")
            nc.scalar.dma_start(
                out=b_bc[:], in_=bp_d[:].unsqueeze(0).broadcast_to([128, C])
            )

            def cast_weights(c):
                nc.vector.tensor_copy(wa_sb[c][:], wstg[c][:, 0: 3 * C])
                nc.vector.tensor_copy(wp_sb[c][:], wstg[c][:, 3 * C: 4 * C])

            def emit_v(b, xT, vb, t):
                psv = psA.tile([128, 512], f32, tag="psA", name=f"psv{b}")
                for c in range(3):
                    nc.tensor.matmul(
                        psv[:, 0:C],
                        lhsT=xT[:, tc_off(t, c): tc_off(t, c) + 128],
                        rhs=wa_sb[c][:, 2 * C: 3 * C],
                        start=(c == 0),
                        stop=(c == 2),
                    )
                nc.vector.tensor_copy(
                    vb[:, t * VSTRIDE: t * VSTRIDE + VSTRIDE]
                    .rearrange("p (h e) -> p h e", e=D + 1)[:, :, 0:D],
                    psv[:, 0:C].rearrange("p (h d) -> p h d", d=D),
                )

            def emit_qk(b, xT, qk, i, m, half):
                psq = psA.tile([128, 512], f32, tag="psA", name=f"psq{b}")
                for c in range(3):
                    nc.tensor.matmul(
                        psq[:],
                        lhsT=wa_sb[c][:, m * 128:(m + 1) * 128],
                        rhs=xT[:].rearrange("p (g k) -> p g k", k=128)
                        [:, half * 12 + c: half * 12 + c + 10: 3, :],
                        start=(c == 0),
                        stop=(c == 2),
                    )
                nc.vector.tensor_copy(
                    qk[:, i * T + half * 512: i * T + half * 512 + 512],
                    psq[:],
                )

            def prep_fillers(b, xf):
                """(vb, qks, early, late) filler lists for batch b.

                Early (deadline 2b):   memset + casts/xbars + all k^T
                                       + q^T half0 + v 0-3.
                Late  (deadline 2b+1): q^T half1 + v 4-7.
                Entry: (pe_cost_ns, closure, deadline, ready_act).
                """
                vb = vbp.tile([128, TT * VSTRIDE], bf16, tag="vb", name=f"vb{b}")
                qks = [qkp.tile([128, 2 * T], bf16, tag=f"qk{pp}",
                                name=f"qk{b}_{pp}") for pp in range(NPAIR)]
                xb = xbp.tile([128, TT * C], bf16, tag="xb", name=f"xb{b}")
                xT = xTp.tile([128, 3 * T], bf16, tag="xT", name=f"xT{b}")

                def head():
                    nc.gpsimd.memset(
                        vb[:].rearrange("p (t h e) -> p t h e", t=TT, e=D + 1)
                        [:, :, :, D:],
                        1.0,
                    )

                def cast_xbar(q):       # 2-tile DVE cast + 2-tile crossbar
                    nc.vector.tensor_copy(
                        xb[:, q * 2 * C:(q + 1) * 2 * C],
                        xf[:, q * 2 * C:(q + 1) * 2 * C],
                    )
                    nc.sync.dma_start_transpose(
                        xT[:, q * 6 * 128:(q + 1) * 6 * 128]
                        .rearrange("p (g k) -> p g k", k=128),
                        xb[:, q * 2 * C:(q + 1) * 2 * C],
                    )
                d0, d1 = 2 * b, 2 * b + 1
                early = [(0.0, head, d0, 0.0)]
                for q in range(4):
                    early.append((200.0, lambda q=q: cast_xbar(q), d0, 0.0))
                qcost = 3 * MM(512)
                vcost = 3 * MM(384)
                eq = []
                for pp in range(NPAIR):
                    eq.append((qcost, lambda pp=pp:
                               emit_qk(b, xT, qks[pp], 0, pp, 0), d0, 0.0))
                    eq.append((qcost, lambda pp=pp:
                               emit_qk(b, xT, qks[pp], 1, 3 + pp, 0),
                               d0, 0.0))
                ev = [(vcost, lambda t=t: emit_v(b, xT, vb, t), d0, 0.0)
                      for t in range(4)]
                while ev or eq:
                    if ev:
                        early.append(ev.pop(0))
                    if eq:
                        early.append(eq.pop(0))
                    if eq:
                        early.append(eq.pop(0))
                late = []
                for pp in range(NPAIR):
                    late.append((qcost, lambda pp=pp:
                                 emit_qk(b, xT, qks[pp], 0, pp, 1), d1, 0.0))
                    late.append((qcost, lambda pp=pp:
                                 emit_qk(b, xT, qks[pp], 1, 3 + pp, 1),
                                 d1 + 0.5, 0.0))
                    late.append((vcost, lambda t=4 + pp:
                                 emit_v(b, xT, vb, t), d1 + 0.5, 0.0))
                late.append((vcost, lambda: emit_v(b, xT, vb, 7), d1 + 0.5, 0.0))
                return vb, qks, early, late

            def emit_proj(b, yT, osb, qc, j):
                t = qc * 4 + j
                pso = psA.tile([128, 512], f32, tag="psA", name=f"pso{b}")
                for c in range(3):
                    nc.tensor.matmul(
                        pso[:, 0:C],
                        lhsT=yT[:, tc_off(t, c): tc_off(t, c) + 128],
                        rhs=wp_sb[c][:],
                        start=(c == 0),
                        stop=(c == 2),
                    )
                nc.vector.tensor_add(osb[:, j * C:(j + 1) * C], pso[:, 0:C],
                                     b_bc[:])
                if j % 2 == 1:
                    hf = j // 2
                    nc.sync.dma_start(
                        out=out_d[b, qc * 512 + hf * 256:
                                  qc * 512 + (hf + 1) * 256, :]
                        .rearrange("(t p) c -> p t c", p=128),
                        in_=osb[:, hf * 2 * C:(hf + 1) * 2 * C]
                        .rearrange("p (t c) -> p t c", t=2),
                    )

            # ---- filler machinery driven by the virtual clocks ----
            fillers = []

            def fill_until():
                while fillers and est["pe"] < est["act"] + VM_MARGIN:
                    hit = None
                    for idx, (cost, f, dl, ready) in enumerate(fillers):
                        if ready <= est["act"]:
                            hit = idx
                            break
                    if hit is None:
                        return
                    cost, f, dl, ready = fillers.pop(hit)
                    f()
                    bump_pe(cost)

            def force_deadline(d):
                rest = []
                for cost, f, dl, ready in fillers:
                    if dl <= d:
                        f()
                        bump_pe(cost)
                    else:
                        rest.append((cost, f, dl, ready))
                fillers[:] = rest

            def drain_fillers():
                while fillers:
                    cost, f, _, _ = fillers.pop(0)
                    f()
                    bump_pe(cost)

            def attn(b, vb, qks):
                """Attention for batch b as ONE flat software-pipelined
                stream: the score/exp steps of all 6 (qc, pair) units run
                back-to-back; each unit's AV tail (j2, j3), softmax
                normalization and the q-chunk's y^T crossbar + proj pushes
                overlap the NEXT unit's score steps, so the PE never drains
                at unit boundaries. Projections are deferred fillers."""
                yT = yTp.tile([128, 3 * T], bf16, tag="yT", name=f"yT{b}")
                yns = {}
                st = {}

                def ycol(j, hh):
                    return (512 if j == 3 else j * 130) + hh * 65

                def emit_scores_exp(qc, p, kt):
                    qk, pT = st["qk"], st["pT"]
                    diag = kt >= qc * 4
                    o = (kt - qc * 4) * 128 if diag else 0
                    pss = psS.tile([128, 1024], f32, tag="psS",
                                   name=f"pss{b}{p}")
                    for hh in range(2):
                        nc.tensor.matmul(
                            pss[:, hh * 512 + o:(hh + 1) * 512],
                            lhsT=qk[hh * 64:(hh + 1) * 64,
                                    T + kt * 128: T + kt * 128 + 128],
                            rhs=qk[hh * 64:(hh + 1) * 64,
                                   qc * 512 + o: qc * 512 + 512],
                            start=True,
                            stop=True,
                        )
                    nc.scalar.activation(
                        pT[:].rearrange("p (h w) -> p h w", h=2)
                        [:, :, kt * 512 + o: (kt + 1) * 512],
                        pss[:].rearrange("p (h w) -> p h w", h=2)
                        [:, :, o:512],
                        Exp,
                        scale=0.125,
                    )
                    bump_pe(2 * MM(512 - o))
                    est["act"] += EXPC(2 * (512 - o))
                    if diag:
                        blk = pT[:].rearrange("p (h w) -> p h w", h=2)[
                            :, :, kt * 512 + o: kt * 512 + o + 128]
                        nc.gpsimd.affine_select(
                            out=blk,
                            in_=blk,
                            compare_op=GE,
                            fill=0.0,
                            base=0,
                            pattern=[[0, 2], [1, 128]],
                            channel_multiplier=-1,
                        )

                def emit_y(u, j):
                    # one accumulation chain per (j, hh) psum region;
                    # chains strictly sequential within a psum bank
                    qc, p, pT, ys = u["qc"], u["p"], u["pT"], u["ys"]
                    ptw = 4 * (qc + 1) * 512
                    qt = qc * 4 + j
                    for hh in range(2):
                        h = 2 * p + hh
                        for k2 in range(qt + 1):
                            nc.tensor.matmul(
                                ys[:, ycol(j, hh): ycol(j, hh) + 65],
                                lhsT=pT[:, hh * ptw + k2 * 512 + j * 128:
                                        hh * ptw + k2 * 512 + j * 128 + 128],
                                rhs=vb[:, k2 * VSTRIDE + h * (D + 1):
                                       k2 * VSTRIDE + (h + 1) * (D + 1)],
                                start=(k2 == 0),
                                stop=(k2 == qt),
                            )
                    bump_pe(2 * (qt + 1) * 38.0)

                def normalize(u):
                    qc, p, ys = u["qc"], u["p"], u["ys"]
                    yn = yns[qc]
                    rc = rcp.tile([128, 8], f32, tag="rc",
                                  name=f"rc{b}{p}{qc}")
                    nc.vector.reciprocal(rc[:, 0:6], ys[:, 64:454:65])
                    nc.vector.reciprocal(rc[:, 6:8], ys[:, 576:706:65])
                    nc.vector.tensor_mul(
                        yn[:, 0: 3 * C]
                        .rearrange("p (j w) -> p j w", j=3)
                        [:, :, 2 * p * 64: 2 * p * 64 + 128]
                        .rearrange("p j (g e) -> p j g e", e=D),
                        ys[:, 0:390]
                        .rearrange("p (j g e) -> p j g e", g=2, e=D + 1)
                        [:, :, :, 0:D],
                        rc[:, 0:6]
                        .rearrange("p (j g) -> p j g", g=2)
                        .unsqueeze(3).broadcast_to([128, 3, 2, D]),
                    )
                    nc.vector.tensor_mul(
                        yn[:, 3 * C + 2 * p * 64: 3 * C + 2 * p * 64 + 128]
                        .rearrange("p (g e) -> p g e", e=D),
                        ys[:, 512:642]
                        .rearrange("p (g e) -> p g e", e=D + 1)[:, :, 0:D],
                        rc[:, 6:8].unsqueeze(2).broadcast_to([128, 2, D]),
                    )
                    if p == NPAIR - 1:
                        finish_qc(qc)

                def finish_qc(qc):
                    # y^T for this qc via two 2-tile crossbars; projections
                    # become deferred fillers (with readiness delay) so the
                    # crossbar latency hides behind the following score steps
                    yn = yns[qc]
                    for hy in range(2):
                        nc.sync.dma_start_transpose(
                            yT[:, (qc * 12 + hy * 6) * 128:
                               (qc * 12 + (hy + 1) * 6) * 128]
                            .rearrange("p (g k) -> p g k", k=128),
                            yn[:, hy * 2 * C:(hy + 1) * 2 * C],
                        )
                    osb = osbp.tile([128, 4 * C], f32, tag="osb",
                                    name=f"osb{b}_{qc}")
                    for j in range(4):
                        fillers.append(
                            (3 * MM(384),
                             lambda j=j, qc=qc, osb=osb:
                             emit_proj(b, yT, osb, qc, j),
                             2 * (b + 1) + qc,
                             est["act"] + PROJ_DELAY)
                        )

                prev = None
                for qc in range(QC):
                    nkt = 4 * (qc + 1)
                    for p in range(NPAIR):
                        if p == 0:
                            force_deadline(2 * b + qc)
                            yns[qc] = ynp.tile([128, 4 * C], bf16,
                                               tag=f"yn{qc}",
                                               name=f"yn{b}_{qc}")
                        st["qk"] = qks[p]
                        st["pT"] = pTp.tile([128, 2 * nkt * 512], bf16,
                                            tag=f"pT{qc}",
                                            name=f"pT{b}_{p}_{qc}")
                        cur = {"qc": qc, "p": p, "pT": st["pT"], "ys": None}
                        for kt in range(nkt):
                            if qc == 1 and p == 0 and kt == 4:
                                force_deadline(2 * b + 1.5)
                            emit_scores_exp(qc, p, kt)
                            fill_until()
                            if kt == 0 and prev is not None:
                                emit_y(prev, 2)
                                emit_y(prev, 3)
                            elif kt == 1:
                                if prev is not None:
                                    normalize(prev)
                                    prev = None
                                # allocate AFTER normalize(prev) so the
                                # single-buffer psY WAR is program-ordered
                                cur["ys"] = psY.tile([128, 1024], f32,
                                                     tag="psY",
                                                     name=f"ys{b}{p}{qc}")
                            if kt - 2 >= qc * 4:
                                emit_y(cur, kt - 2 - qc * 4)
                        prev = cur
                # batch tail
                emit_y(prev, 2)
                fill_until()
                emit_y(prev, 3)
                normalize(prev)

            # ---- schedule ----
            vb0, qks0, early0, late0 = prep_fillers(0, xf0)
            for c in range(3):
                early0.insert(5 + c,
                              (0.0, lambda c=c: cast_weights(c), 0, 0.0))
            for cost, f, _, _ in early0:
                f()
                est["pe"] += cost
            xf_next = load(1)

            vb_cur, qks_cur, late_cur = vb0, qks0, late0
            for b in range(BPC):
                if b + 1 < BPC:
                    vb_nxt, qks_nxt, early_n, late_n = prep_fillers(b + 1, xf_next)
                else:
                    early_n = []
                # interleave late(b) between the cast/xbar entries of
                # early(b+1) so the first v/qk of b+1 never waits on a
                # just-issued crossbar
                merged = []
                la, ea = list(late_cur), list(early_n)
                while la or ea:
                    if ea:
                        merged.append(ea.pop(0))
                    if la:
                        merged.append(la.pop(0))
                fillers.extend(merged)
                if b + 1 < BPC - 1:
                    # next batch's late prep is also fair game this batch --
                    # it shifts supply toward each batch's filler-starved
                    # qc1 tail (the last batch keeps its own late set)
                    fillers.extend(late_n)
                    late_n = []
                if b + 2 < BPC:
                    xf_next = load(b + 2)
                attn(b, vb_cur, qks_cur)
                if b + 1 < BPC:
                    vb_cur, qks_cur, late_cur = vb_nxt, qks_nxt, late_n
            drain_fillers()

    nc.finalize()
    return nc


def _run(inputs, trace=False, **kw):
    from concourse.bass_utils import run_bass_kernel_spmd

    if "nc" not in _nc_cache:
        _nc_cache["nc"] = _build_nc()
    nc = _nc_cache["nc"]

    x = np.ascontiguousarray(np.asarray(inputs["x"], dtype=np.float32))
    wa = np.ascontiguousarray(np.asarray(inputs["W_attn"], dtype=np.float32))
    wp = np.ascontiguousarray(np.asarray(inputs["W_proj"], dtype=np.float32))
    bp = np.ascontiguousarray(np.asarray(inputs["b_proj"], dtype=np.float32))

    in_maps = [
        {"x": x[i * BPC:(i + 1) * BPC], "W_attn": wa, "W_proj": wp, "b_proj": bp}
        for i in range(NCORES)
    ]
    res = run_bass_kernel_spmd(nc, in_maps, list(range(NCORES)), trace=trace, **kw)
    out = np.concatenate([res.results[i]["out"] for i in range(NCORES)], axis=0)
    return out, res


def kernel(**inputs) -> np.ndarray:
    out, _ = _run(inputs, trace=False)
    return out


# revision 19
# speedup vs baseline: 1.1581x; 1.1581x over previous
"""Multi-head causal self-attention block for Trainium2, data-parallel over 8 cores.

Reference computation (per batch b of x [B=32, T=1024, C=384]):
    qkv = x @ W_attn;  q,k,v heads (H=6, D=64)
    y   = softmax(causal(q k^T / sqrt(D))) @ v
    out = y @ W_proj + b_proj
Sharding: batch dim 32 -> 4 per core, weights replicated, no collectives.

v6 design (evolves v3..v5; bf16 matmul operands, f32 psum accumulation):
  - Every dma_start costs ~600ns of ISSUING-sequencer occupancy, so DMAs are
    batched hard: x loads are 2 DMAs/batch, x^T and y^T crossbar transposes
    are 2 calls/batch each (covering 4 token tiles at a time), and output
    stores are 2 gpsimd SWDGE DMAs/batch (keeping the sync queue short).
  - x^T / y^T live in T-MAJOR chunk layout (chunk index = t*3 + c) so a
    4-tile crossbar call writes one contiguous [128, 12, 128] region; the
    PE does no transposes at all and the DVE does no transpose evictions.
  - x is cast f32->bf16 on the DVE in 768-col quarters (gpsimd tensor ops
    are ~4x slower and its in-order queue must stay clear for the causal
    affine_selects).
  - q^T,k^T per head-pair [128, T] (W_attn slices stationary); v natural
    [tok, 384] per k-tile with a bf16 ones column per head (softmax
    denominator rides the AV matmul for free).
  - scores transposed s^T[k,q] per 512-wide q chunk, exp on ACT (scale=1/8)
    into bf16 pT; causal diagonal fixed by gpsimd affine_select.
  - AV in NATURAL layout: out[q, d+1] accumulated per 128-q chunk; softmax
    normalization = strided DVE reciprocal + broadcast-AP multiplies.
  - proj runs as deferred filler work (with a readiness delay) so the y^T
    crossbar latency hides behind the next q-chunk's score phase; bias is
    added on the DVE eviction into a 4-tile output buffer.
  - VIRTUAL-CLOCK INTERLEAVE: emission tracks estimated cumulative PE and
    ACT busy-ns; filler chunks (next batch's cast/xbar/v/qk, deferred
    proj) are inserted whenever the PE stream would fall behind the ACT
    stream, keeping the in-order PE queue dense so the tensor engine
    stays ramped at its top p-state. Fillers carry deadline tags (global
    qc index; force-emitted when their consumer phase begins) and a
    readiness gate (min ACT-clock before emission). Per-batch prep is
    split into an "early" part (needed before the batch's qc0 attention)
    and a "late" part (only needed by qc1), widening the filler window.
"""

import sys

if "/opt/trn_rl_repo" not in sys.path:
    sys.path.insert(0, "/opt/trn_rl_repo")

import numpy as np

B, T, C = 32, 1024, 384
H, D = 6, 64
NCORES = 8
BPC = B // NCORES          # batches per core
NPAIR = H // 2             # head pairs
TT = T // 128              # token tiles per batch (8)
QC = T // 512              # q chunks per batch (2)
VSTRIDE = H * (D + 1)      # 390: per-token-tile v_aug row width

VM_MARGIN = 500.0          # ns of PE lead to maintain over ACT
VM_CAP = 1500.0            # max bankable PE lead (queue depth model)
PROJ_DELAY = 2500.0        # ns of ACT-clock before a deferred proj is ready

_nc_cache = {}


def _build_nc():
    import concourse.mybir as mybir
    from concourse import bacc
    from concourse.tile import TileContext

    f32 = mybir.dt.float32
    bf16 = mybir.dt.bfloat16
    Exp = mybir.ActivationFunctionType.Exp
    GE = mybir.AluOpType.is_ge

    nc = bacc.Bacc("TRN2", target_bir_lowering=False, debug=False, num_devices=NCORES)

    x_d = nc.declare_dram_parameter("x", [BPC, T, C], f32, isOutput=False)
    wa_d = nc.declare_dram_parameter("W_attn", [C, 3 * C], f32, isOutput=False)
    wp_d = nc.declare_dram_parameter("W_proj", [C, C], f32, isOutput=False)
    bp_d = nc.declare_dram_parameter("b_proj", [C], f32, isOutput=False)
    out_d = nc.declare_dram_parameter("out", [BPC, T, C], f32, isOutput=True)

    # virtual clocks (ns) for PE / ACT emission balancing
    est = {"pe": 0.0, "act": 0.0}

    def MM(n):                      # matmul cost, cols n
        return n / 2.4 + 10.0

    def EXPC(n):                    # ACT exp cost, free-elems n
        return n * 1.0 + 250.0

    def bump_pe(cost):
        est["pe"] = min(est["pe"] + cost, est["act"] + VM_CAP)

    def tc_off(t, c):               # t-major chunk offset into xT / yT
        return (t * 3 + c) * 128

    with TileContext(nc) as tc:
        with (
            tc.tile_pool(name="const", bufs=1) as const,
            tc.tile_pool(name="xf", bufs=2) as xfp,
            tc.tile_pool(name="xb", bufs=2) as xbp,
            tc.tile_pool(name="xT", bufs=2) as xTp,
            tc.tile_pool(name="qk", bufs=2) as qkp,
            tc.tile_pool(name="vb", bufs=2) as vbp,
            tc.tile_pool(name="pT", bufs=2) as pTp,
            tc.tile_pool(name="yn", bufs=2) as ynp,
            tc.tile_pool(name="yT", bufs=2) as yTp,
            tc.tile_pool(name="rc", bufs=4) as rcp,
            tc.tile_pool(name="osb", bufs=2) as osbp,
            tc.tile_pool(name="psA", bufs=2, space="PSUM") as psA,   # 1 bank each
            tc.tile_pool(name="psS", bufs=2, space="PSUM") as psS,   # 2 banks each
            tc.tile_pool(name="psY", bufs=1, space="PSUM") as psY,   # 2 banks
        ):
            def load(b):
                """x[b] f32 DRAM -> SBUF staging, two 4-tile DMAs."""
                xf = xfp.tile([128, TT * C], f32, tag="xf", name=f"xf{b}")
                for hf in range(2):
                    nc.sync.dma_start(
                        out=xf[:, hf * 4 * C:(hf + 1) * 4 * C]
                        .rearrange("p (t c) -> p t c", t=4),
                        in_=x_d[b, hf * 512:(hf + 1) * 512, :]
                        .rearrange("(t p) c -> p t c", p=128),
                    )
                return xf

            # ---- prologue: x0 first (longest chain), then weight DMAs;
            # weight CASTS are emitted inside early0 after the first x
            # casts so the DVE starts the x chain as soon as data lands ----
            xf0 = load(0)

            wa_sb = []
            wp_sb = []
            wstg = []
            for c in range(3):
                wf = const.tile([128, 4 * C], f32, tag=f"wf{c}")
                nc.scalar.dma_start(out=wf[:, 0: 3 * C],
                                    in_=wa_d[c * 128:(c + 1) * 128, :])
                nc.scalar.dma_start(out=wf[:, 3 * C: 4 * C],
                                    in_=wp_d[c * 128:(c + 1) * 128, :])
                wstg.append(wf)
            for c in range(3):
                w = const.tile([128, 3 * C], bf16, tag=f"wa{c}")
                wa_sb.append(w)
                p = const.tile([128, C], bf16, tag=f"wp{c}")
                wp_sb.append(p)
            b_bc = const.tile([128, C], f32, tag="bbc")
            nc.scalar.dma_start(
                out=b_bc[:], in_=bp_d[:].unsqueeze(0).broadcast_to([128, C])
            )

            def cast_weights(c):
                nc.vector.tensor_copy(wa_sb[c][:], wstg[c][:, 0: 3 * C])
                nc.vector.tensor_copy(wp_sb[c][:], wstg[c][:, 3 * C: 4 * C])

            def emit_v(b, xT, vb, t):
                psv = psA.tile([128, 512], f32, tag="psA", name=f"psv{b}")
                for c in range(3):
                    nc.tensor.matmul(
                        psv[:, 0:C],
                        lhsT=xT[:, tc_off(t, c): tc_off(t, c) + 128],
                        rhs=wa_sb[c][:, 2 * C: 3 * C],
                        start=(c == 0),
                        stop=(c == 2),
                    )
                nc.vector.tensor_copy(
                    vb[:, t * VSTRIDE: t * VSTRIDE + VSTRIDE]
                    .rearrange("p (h e) -> p h e", e=D + 1)[:, :, 0:D],
                    psv[:, 0:C].rearrange("p (h d) -> p h d", d=D),
                )

            def emit_qk(b, xT, qk, i, m, half):
                psq = psA.tile([128, 512], f32, tag="psA", name=f"psq{b}")
                for c in range(3):
                    nc.tensor.matmul(
                        psq[:],
                        lhsT=wa_sb[c][:, m * 128:(m + 1) * 128],
                        rhs=xT[:].rearrange("p (g k) -> p g k", k=128)
                        [:, half * 12 + c: half * 12 + c + 10: 3, :],
                        start=(c == 0),
                        stop=(c == 2),
                    )
                nc.vector.tensor_copy(
                    qk[:, i * T + half * 512: i * T + half * 512 + 512],
                    psq[:],
                )

            def prep_fillers(b, xf):
                """(vb, qks, early, late) filler lists for batch b.

                Early (deadline 2b):   memset + casts/xbars + all k^T
                                       + q^T half0 + v 0-3.
                Late  (deadline 2b+1): q^T half1 + v 4-7.
                Entry: (pe_cost_ns, closure, deadline, ready_act).
                """
                vb = vbp.tile([128, TT * VSTRIDE], bf16, tag="vb", name=f"vb{b}")
                qks = [qkp.tile([128, 2 * T], bf16, tag=f"qk{pp}",
                                name=f"qk{b}_{pp}") for pp in range(NPAIR)]
                xb = xbp.tile([128, TT * C], bf16, tag="xb", name=f"xb{b}")
                xT = xTp.tile([128, 3 * T], bf16, tag="xT", name=f"xT{b}")

                def head():
                    nc.gpsimd.memset(
                        vb[:].rearrange("p (t h e) -> p t h e", t=TT, e=D + 1)
                        [:, :, :, D:],
                        1.0,
                    )

                def cast_xbar(q):       # 2-tile DVE cast + 2-tile crossbar
                    nc.vector.tensor_copy(
                        xb[:, q * 2 * C:(q + 1) * 2 * C],
                        xf[:, q * 2 * C:(q + 1) * 2 * C],
                    )
                    nc.sync.dma_start_transpose(
                        xT[:, q * 6 * 128:(q + 1) * 6 * 128]
                        .rearrange("p (g k) -> p g k", k=128),
                        xb[:, q * 2 * C:(q + 1) * 2 * C],
                    )
                d0, d1 = 2 * b, 2 * b + 1
                RDY = est["act"] + 3000.0
                early = [(0.0, head, d0, 0.0)]
                for q in range(4):
                    early.append((200.0, lambda q=q: cast_xbar(q), d0, 0.0))
                qcost = 3 * MM(512)
                vcost = 3 * MM(384)
                eq = []
                for pp in range(NPAIR):
                    eq.append((qcost, lambda pp=pp:
                               emit_qk(b, xT, qks[pp], 0, pp, 0), d0, RDY))
                    for half in range(2):
                        eq.append((qcost, lambda pp=pp, half=half:
                                   emit_qk(b, xT, qks[pp], 1, 3 + pp, half),
                                   d0, RDY))
                ev = [(vcost, lambda t=t: emit_v(b, xT, vb, t), d0, RDY)
                      for t in range(4)]
                while ev or eq:
                    if ev:
                        early.append(ev.pop(0))
                    if eq:
                        early.append(eq.pop(0))
                    if eq:
                        early.append(eq.pop(0))
                late = []
                for pp in range(NPAIR):
                    late.append((qcost, lambda pp=pp:
                                 emit_qk(b, xT, qks[pp], 0, pp, 1), d1, 0.0))
                    late.append((vcost, lambda t=4 + pp:
                                 emit_v(b, xT, vb, t), d1, 0.0))
                late.append((vcost, lambda: emit_v(b, xT, vb, 7), d1, 0.0))
                return vb, qks, early, late

            def emit_proj(b, yT, osb, qc, j):
                t = qc * 4 + j
                pso = psA.tile([128, 512], f32, tag="psA", name=f"pso{b}")
                for c in range(3):
                    nc.tensor.matmul(
                        pso[:, 0:C],
                        lhsT=yT[:, tc_off(t, c): tc_off(t, c) + 128],
                        rhs=wp_sb[c][:],
                        start=(c == 0),
                        stop=(c == 2),
                    )
                nc.vector.tensor_add(osb[:, j * C:(j + 1) * C], pso[:, 0:C],
                                     b_bc[:])
                if j % 2 == 1:
                    hf = j // 2
                    nc.sync.dma_start(
                        out=out_d[b, qc * 512 + hf * 256:
                                  qc * 512 + (hf + 1) * 256, :]
                        .rearrange("(t p) c -> p t c", p=128),
                        in_=osb[:, hf * 2 * C:(hf + 1) * 2 * C]
                        .rearrange("p (t c) -> p t c", t=2),
                    )

            # ---- filler machinery driven by the virtual clocks ----
            fillers = []

            def fill_until():
                while fillers and est["pe"] < est["act"] + VM_MARGIN:
                    hit = None
                    for idx, (cost, f, dl, ready) in enumerate(fillers):
                        if ready <= est["act"]:
                            hit = idx
                            break
                    if hit is None:
                        return
                    cost, f, dl, ready = fillers.pop(hit)
                    f()
                    bump_pe(cost)

            def force_deadline(d):
                rest = []
                for cost, f, dl, ready in fillers:
                    if dl <= d:
                        f()
                        bump_pe(cost)
                    else:
                        rest.append((cost, f, dl, ready))
                fillers[:] = rest

            def drain_fillers():
                while fillers:
                    cost, f, _, _ = fillers.pop(0)
                    f()
                    bump_pe(cost)

            def attn(b, vb, qks):
                """Attention for batch b as ONE flat software-pipelined
                stream: the score/exp steps of all 6 (qc, pair) units run
                back-to-back; each unit's AV tail (j2, j3), softmax
                normalization and the q-chunk's y^T crossbar + proj pushes
                overlap the NEXT unit's score steps, so the PE never drains
                at unit boundaries. Projections are deferred fillers."""
                yT = yTp.tile([128, 3 * T], bf16, tag="yT", name=f"yT{b}")
                yns = {}
                st = {}

                def ycol(j, hh):
                    return (512 if j == 3 else j * 130) + hh * 65

                def emit_scores_exp(qc, p, kt):
                    qk, pT = st["qk"], st["pT"]
                    diag = kt >= qc * 4
                    o = (kt - qc * 4) * 128 if diag else 0
                    pss = psS.tile([128, 1024], f32, tag="psS",
                                   name=f"pss{b}{p}")
                    for hh in range(2):
                        nc.tensor.matmul(
                            pss[:, hh * 512 + o:(hh + 1) * 512],
                            lhsT=qk[hh * 64:(hh + 1) * 64,
                                    T + kt * 128: T + kt * 128 + 128],
                            rhs=qk[hh * 64:(hh + 1) * 64,
                                   qc * 512 + o: qc * 512 + 512],
                            start=True,
                            stop=True,
                        )
                    nc.scalar.activation(
                        pT[:].rearrange("p (h w) -> p h w", h=2)
                        [:, :, kt * 512 + o: (kt + 1) * 512],
                        pss[:].rearrange("p (h w) -> p h w", h=2)
                        [:, :, o:512],
                        Exp,
                        scale=0.125,
                    )
                    bump_pe(2 * MM(512 - o))
                    est["act"] += EXPC(2 * (512 - o))
                    if diag:
                        blk = pT[:].rearrange("p (h w) -> p h w", h=2)[
                            :, :, kt * 512 + o: kt * 512 + o + 128]
                        nc.gpsimd.affine_select(
                            out=blk,
                            in_=blk,
                            compare_op=GE,
                            fill=0.0,
                            base=0,
                            pattern=[[0, 2], [1, 128]],
                            channel_multiplier=-1,
                        )

                def emit_y(u, j):
                    # one accumulation chain per (j, hh) psum region;
                    # chains strictly sequential within a psum bank
                    qc, p, pT, ys = u["qc"], u["p"], u["pT"], u["ys"]
                    ptw = 4 * (qc + 1) * 512
                    qt = qc * 4 + j
                    for hh in range(2):
                        h = 2 * p + hh
                        for k2 in range(qt + 1):
                            nc.tensor.matmul(
                                ys[:, ycol(j, hh): ycol(j, hh) + 65],
                                lhsT=pT[:, hh * ptw + k2 * 512 + j * 128:
                                        hh * ptw + k2 * 512 + j * 128 + 128],
                                rhs=vb[:, k2 * VSTRIDE + h * (D + 1):
                                       k2 * VSTRIDE + (h + 1) * (D + 1)],
                                start=(k2 == 0),
                                stop=(k2 == qt),
                            )
                    bump_pe(2 * (qt + 1) * 38.0)

                def normalize(u):
                    qc, p, ys = u["qc"], u["p"], u["ys"]
                    yn = yns[qc]
                    rc = rcp.tile([128, 8], f32, tag="rc",
                                  name=f"rc{b}{p}{qc}")
                    nc.vector.reciprocal(rc[:, 0:6], ys[:, 64:454:65])
                    nc.vector.reciprocal(rc[:, 6:8], ys[:, 576:706:65])
                    nc.vector.tensor_mul(
                        yn[:, 0: 3 * C]
                        .rearrange("p (j w) -> p j w", j=3)
                        [:, :, 2 * p * 64: 2 * p * 64 + 128]
                        .rearrange("p j (g e) -> p j g e", e=D),
                        ys[:, 0:390]
                        .rearrange("p (j g e) -> p j g e", g=2, e=D + 1)
                        [:, :, :, 0:D],
                        rc[:, 0:6]
                        .rearrange("p (j g) -> p j g", g=2)
                        .unsqueeze(3).broadcast_to([128, 3, 2, D]),
                    )
                    nc.vector.tensor_mul(
                        yn[:, 3 * C + 2 * p * 64: 3 * C + 2 * p * 64 + 128]
                        .rearrange("p (g e) -> p g e", e=D),
                        ys[:, 512:642]
                        .rearrange("p (g e) -> p g e", e=D + 1)[:, :, 0:D],
                        rc[:, 6:8].unsqueeze(2).broadcast_to([128, 2, D]),
                    )
                    if p == NPAIR - 1:
                        finish_qc(qc)

                def finish_qc(qc):
                    # y^T for this qc via two 2-tile crossbars; projections
                    # become deferred fillers (with readiness delay) so the
                    # crossbar latency hides behind the following score steps
                    yn = yns[qc]
                    for hy in range(2):
                        nc.sync.dma_start_transpose(
                            yT[:, (qc * 12 + hy * 6) * 128:
                               (qc * 12 + (hy + 1) * 6) * 128]
                            .rearrange("p (g k) -> p g k", k=128),
                            yn[:, hy * 2 * C:(hy + 1) * 2 * C],
                        )
                    osb = osbp.tile([128, 4 * C], f32, tag="osb",
                                    name=f"osb{b}_{qc}")
                    for j in range(4):
                        fillers.append(
                            (3 * MM(384),
                             lambda j=j, qc=qc, osb=osb:
                             emit_proj(b, yT, osb, qc, j),
                             2 * (b + 1) + qc,
                             est["act"] + PROJ_DELAY)
                        )

                prev = None
                for qc in range(QC):
                    nkt = 4 * (qc + 1)
                    for p in range(NPAIR):
                        if p == 0:
                            force_deadline(2 * b + qc)
                            yns[qc] = ynp.tile([128, 4 * C], bf16,
                                               tag=f"yn{qc}",
                                               name=f"yn{b}_{qc}")
                        st["qk"] = qks[p]
                        st["pT"] = pTp.tile([128, 2 * nkt * 512], bf16,
                                            tag=f"pT{qc}",
                                            name=f"pT{b}_{p}_{qc}")
                        cur = {"qc": qc, "p": p, "pT": st["pT"], "ys": None}
                        for kt in range(nkt):
                            emit_scores_exp(qc, p, kt)
                            fill_until()
                            if kt == 0 and prev is not None:
                                emit_y(prev, 2)
                                emit_y(prev, 3)
                            elif kt == 1:
                                if prev is not None:
                                    normalize(prev)
                                    prev = None
                                # allocate AFTER normalize(prev) so the
                                # single-buffer psY WAR is program-ordered
                                cur["ys"] = psY.tile([128, 1024], f32,
                                                     tag="psY",
                                                     name=f"ys{b}{p}{qc}")
                            if kt - 2 >= qc * 4:
                                emit_y(cur, kt - 2 - qc * 4)
                        prev = cur
                # batch tail
                emit_y(prev, 2)
                fill_until()
                emit_y(prev, 3)
                normalize(prev)

            # ---- schedule ----
            vb0, qks0, early0, late0 = prep_fillers(0, xf0)
            for c in range(3):
                early0.insert(5 + c,
                              (0.0, lambda c=c: cast_weights(c), 0, 0.0))
            for cost, f, _, _ in early0:
                f()
                est["pe"] += cost
            xf_next = load(1)

            vb_cur, qks_cur, late_cur = vb0, qks0, late0
            for b in range(BPC):
                if b + 1 < BPC:
                    vb_nxt, qks_nxt, early_n, late_n = prep_fillers(b + 1, xf_next)
                else:
                    early_n = []
                # interleave late(b) between the cast/xbar entries of
                # early(b+1) so the first v/qk of b+1 never waits on a
                # just-issued crossbar
                merged = []
                la, ea = list(late_cur), list(early_n)
                while la or ea:
                    if ea:
                        merged.append(ea.pop(0))
                    if la:
                        merged.append(la.pop(0))
                fillers.extend(merged)
                if b + 1 < BPC - 1:
                    # next batch's late prep is also fair game this batch --
                    # it shifts supply toward each batch's filler-starved
                    # qc1 tail (the last batch keeps its own late set)
                    fillers.extend(late_n)
                    late_n = []
                if b + 2 < BPC:
                    xf_next = load(b + 2)
                attn(b, vb_cur, qks_cur)
                if b + 1 < BPC:
                    vb_cur, qks_cur, late_cur = vb_nxt, qks_nxt, late_n
            drain_fillers()

    nc.finalize()
    return nc


def _run(inputs, trace=False, **kw):
    from concourse.bass_utils import run_bass_kernel_spmd

    if "nc" not in _nc_cache:
        _nc_cache["nc"] = _build_nc()
    nc = _nc_cache["nc"]

    x = np.ascontiguousarray(np.asarray(inputs["x"], dtype=np.float32))
    wa = np.ascontiguousarray(np.asarray(inputs["W_attn"], dtype=np.float32))
    wp = np.ascontiguousarray(np.asarray(inputs["W_proj"], dtype=np.float32))
    bp = np.ascontiguousarray(np.asarray(inputs["b_proj"], dtype=np.float32))

    in_maps = [
        {"x": x[i * BPC:(i + 1) * BPC], "W_attn": wa, "W_proj": wp, "b_proj": bp}
        for i in range(NCORES)
    ]
    res = run_bass_kernel_spmd(nc, in_maps, list(range(NCORES)), trace=trace, **kw)
    out = np.concatenate([res.results[i]["out"] for i in range(NCORES)], axis=0)
    return out, res


def kernel(**inputs) -> np.ndarray:
    out, _ = _run(inputs, trace=False)
    return out
